# revision 54
# baseline (speedup 1.0000x reference)
"""Trainium2 Bass kernel for RoPE + GQA causal attention (B=1, S=2048, HID=2048,
NH=16, NKV=4, HD=128), tensor-parallel over heads across 8 NeuronCores.

Core c computes q heads {2c, 2c+1} and kv head c//2 plus the matching wo
input-dim slice; each core emits a partial [S, HID] (f16) output and the host
sums the 8 partials.

v9 design notes (vs the 181us v2 baseline):
  - 512-column compute chunks (proj j -> rope -> attn j pipelined), but x
    streams in 1024-column host-blocked supertiles so every DMA descriptor
    carries a full 2KB partition line.
  - DMA instruction count minimized (~40 total): each dma_start costs ~620ns
    of sequencer issue time regardless of size, so kc tiles ride grouped
    multi-kc DMAs and the 64 output stores merge into 16 wide per-st stores.
  - two HW-DGE queues stream in parallel (xt on sync, x8/consts on scalar);
    gpsimd issues NO DMAs (software-DGE: 600ns/issue + ~10us teardown drain).
  - per chunk, the fp8 DoubleRow k/q pass + rope run BEFORE the bf16 v pass,
    so the attention-critical path rides the small fp8 stream.
  - v is computed directly in [s, d] layout (lhsT = x tile, rhs = wv tile,
    4 psum series sharing one bank: ONE start=True, has_written handles the
    rest), removing all DMA transposes.
  - softmax den/recip/broadcast run at attn-chunk end; only the normalize
    mul + out-proj release ride the next chunk, so the PE queue head never
    blocks ready work; op-unit psum copies alternate ACT/DVE strictly.
  - warm-up dummy matmuls fill DMA-paced front slivers (HAM clock stays up).
  - carried over from v2: fp8 DoubleRow projections with W8S weight prescale,
    paired-head [128,2,512] tiles, bf16 pairwise den tree + ones-matmul,
    causal col-skip + 0/1 triangle mask, out-projection work-unit drain.
"""

import os
import sys
from contextlib import ExitStack

for _p in ("/opt/trn_rl_repo", "/root/.axon_site/_ro/trn_rl_repo"):
    if os.path.isdir(_p) and _p not in sys.path:
        sys.path.append(_p)

import ml_dtypes
import numpy as np

import concourse.bass as bass
import concourse.mybir as mybir
import concourse.tile as tile
from concourse import bacc, bass_utils

S, HID, NH, NKV, HD = 2048, 2048, 16, 4, 128
HH = HD // 2  # 64
NCORES = 8
QH = NH // NCORES  # 2 q heads per core
SCALE = float(1.0 / np.sqrt(HD))

F32 = mybir.dt.float32
BF16 = mybir.dt.bfloat16
F16 = mybir.dt.float16
F8 = mybir.dt.float8e4
NPBF = ml_dtypes.bfloat16
NPF8 = ml_dtypes.float8_e4m3fn

# q/k projections run in fp8 (DoubleRow); weights are pre-scaled by W8S on the
# host, so scores carry W8S^2 and the exp scale divides it back out
W8S = 32.0

NKC = HID // 128   # 16 contraction chunks
NK2 = HID // 256   # 8 double-row contraction chunks
SC = 512           # s-chunk width
NC = S // SC       # 4 chunks
NB = SC // 128     # 4 128-blocks per chunk
# attention chunks: (col0, width); tail kept short (last chunk 128)
ATTN_CHUNKS = [(0, 512), (512, 512), (1024, 512), (1536, 384), (1920, 128)]


def build_graph():
    nc = bacc.Bacc(trn_type="TRN2", enable_partition_id=False)

    # host-blocked streams in 1024-col supertiles, PER-PARTITION-MAJOR: each
    # partition's slice of a DMA is one long contiguous HBM run, so the DGE
    # emits 8-16KB descriptors instead of 2KB ones (expansion rate was the
    # front-fill bottleneck)
    xt_d = nc.dram_tensor("xt", [2, 128, NKC, 2 * SC], BF16,
                          kind="ExternalInput")
    x8_d = nc.dram_tensor("x8", [2, 128, NK2, 2, 2 * SC], F8,
                          kind="ExternalInput")
    wv_d = nc.dram_tensor("wvt", [128, NKC, HD], BF16, kind="ExternalInput")
    wqk8_d = nc.dram_tensor("wqk8", [128, NK2, 2, 3 * HD], F8,
                            kind="ExternalInput")
    wo_d = nc.dram_tensor("wot", [128, QH, HID], BF16, kind="ExternalInput")
    c1d = nc.dram_tensor("c1", [HD, S], BF16, kind="ExternalInput")
    c2d = nc.dram_tensor("c2", [HD, S], BF16, kind="ExternalInput")
    r1d = nc.dram_tensor("r1t", [HD, HD], BF16, kind="ExternalInput")
    r2d = nc.dram_tensor("r2t", [HD, HD], BF16, kind="ExternalInput")
    # blocked output: [st, 128, 2048] f16 (one wide store per 128-row block)
    outd = nc.dram_tensor("out", [S // 128, 128, HID], F16,
                          kind="ExternalOutput")


    with tile.TileContext(nc) as tc, ExitStack() as ctx:
        # ---------- pools
        consts = ctx.enter_context(tc.tile_pool(name="consts", bufs=1))
        persist = ctx.enter_context(tc.tile_pool(name="persist", bufs=1))
        xtp = ctx.enter_context(tc.tile_pool(name="xtp", bufs=2))
        x8p = ctx.enter_context(tc.tile_pool(name="x8p", bufs=2))
        rawp = ctx.enter_context(tc.tile_pool(name="rawp", bufs=3))
        t12p = ctx.enter_context(tc.tile_pool(name="t12p", bufs=1))
        ep = ctx.enter_context(tc.tile_pool(name="ep", bufs=6))
        trp = ctx.enter_context(tc.tile_pool(name="trp", bufs=5))
        obp = ctx.enter_context(tc.tile_pool(name="obp", bufs=3))
        dbp = ctx.enter_context(tc.tile_pool(name="dbp", bufs=2))
        # PSUM: 4 + 2 + 2 = 8 banks exactly
        p1 = ctx.enter_context(
            tc.tile_pool(name="p1", bufs=2, space="PSUM"))   # [128,2,512]f32 x2
        p2 = ctx.enter_context(
            tc.tile_pool(name="p2", bufs=1, space="PSUM"))   # [128,2,512]f32 x1
        p3 = ctx.enter_context(
            tc.tile_pool(name="p3", bufs=2, space="PSUM"))   # [128,512]f32 x2

        # ---------- persistent SBUF
        q_pair = persist.tile([128, QH, S], BF16, tag="q_pair")
        kT = persist.tile([128, S], BF16, tag="kT")
        v_sd = persist.tile([128, S // 128, HD], BF16, tag="v_sd")
        ao_pair = persist.tile([128, QH, S], BF16, tag="ao_pair")
        wv_sb = persist.tile([128, NKC, HD], BF16, tag="wv_sb")
        wqk8_sb = persist.tile([128, NK2, 2, 3 * HD], F8, tag="wqk8_sb")
        wo_sb = persist.tile([128, QH, HID], BF16, tag="wo_sb")
        c1h = [persist.tile([128, S // 2], BF16, tag=f"c1_{i}",
                            name=f"c1_{i}") for i in range(2)]
        c2h = [persist.tile([128, S // 2], BF16, tag=f"c2_{i}",
                            name=f"c2_{i}") for i in range(2)]

        r1_sb = consts.tile([128, 128], BF16)
        r2_sb = consts.tile([128, 128], BF16)
        ones_col = consts.tile([128, 1], BF16)
        nc.vector.memset(ones_col, 1.0)
        # 0/1 lower-triangle keep mask (keep where col >= row), both heads
        tri01 = consts.tile([128, QH, 128], BF16)
        nc.gpsimd.memset(tri01, 1.0)
        nc.gpsimd.affine_select(
            out=tri01,
            in_=tri01,
            compare_op=mybir.AluOpType.is_ge,
            fill=0.0,
            base=0,
            pattern=[[0, QH], [1, 128]],  # value = col
            channel_multiplier=-1,        # - row
        )

        # ---------- x supertile SBUF tiles + initial DMAs (sup 0 + weights)
        xt_sb = {}
        x8_sb = {}

        def alloc_sup(sup):
            xt_sb[sup] = xtp.tile([128, NKC, 2 * SC], BF16, tag="xt",
                                  name=f"xt{sup}")
            x8_sb[sup] = x8p.tile([128, NK2, 2, 2 * SC], F8, tag="x8",
                                  name=f"x8{sup}")

        # Every dma_start costs ~620ns of sequencer issue time regardless of
        # size, so DMAs are batched into as few instructions as possible.
        # gpsimd additionally must issue NO DMAs (software-DGE + 10us drain).
        alloc_sup(0)
        # scalar feeds the q/k pass (x8) plus rope consts; sync feeds the
        # (now later) v pass. Two DGE queues stream in parallel.
        nc.scalar.dma_start(r1_sb, r1d[:, :])
        nc.scalar.dma_start(r2_sb, r2d[:, :])
        nc.scalar.dma_start(wqk8_sb, wqk8_d[:, :, :, :])
        nc.sync.dma_start(c1h[0], c1d[:, 0:1024])
        nc.scalar.dma_start(x8_sb[0][:, 0:4, :, :], x8_d[0, :, 0:4, :, :])
        nc.sync.dma_start(xt_sb[0][:, 0:8, :], xt_d[0, :, 0:8, :])
        nc.scalar.dma_start(x8_sb[0][:, 4:8, :, :], x8_d[0, :, 4:8, :, :])
        nc.scalar.dma_start(c2h[0], c2d[:, 0:1024])
        nc.sync.dma_start(xt_sb[0][:, 8:16, :], xt_d[0, :, 8:16, :])
        nc.scalar.dma_start(wv_sb, wv_d[:, :, :])
        nc.scalar.dma_start(wo_sb, wo_d[:, :, :])

        # ---------- PE warm-up: dummy matmuls that fill DMA-paced slivers in
        # the front so the HAM clock ramps to 2.4GHz before the dense phase
        warm_tile = [None]

        def warm(n):
            if warm_tile[0] is None:
                warm_tile[0] = p3.tile([128, 512], F32, tag="oc", name="warm")
            for _ in range(n):
                nc.tensor.matmul(warm_tile[0][:, 0:128], r1_sb, r1_sb,
                                 start=True, stop=True)

        # ---------- dribbled prefetch queue (issued on sync only)
        pending_dmas = []
        drip_i = [0]

        def queue_sup_prefetch(sup):
            alloc_sup(sup)
            for g in range(4):
                pending_dmas.append((xt_sb[sup][:, 4 * g:4 * g + 4, :],
                                     xt_d[sup, :, 4 * g:4 * g + 4, :]))
                if g % 2 == 1:
                    k0 = 2 * g - 2
                    pending_dmas.append((x8_sb[sup][:, k0:k0 + 4, :, :],
                                         x8_d[sup, :, k0:k0 + 4, :, :]))

        def queue_wo_prefetch():
            pending_dmas.append((c1h[1], c1d[:, 1024:2048]))
            pending_dmas.append((c2h[1], c2d[:, 1024:2048]))

        def drip(n):
            for _ in range(n):
                if not pending_dmas:
                    return
                dst, src = pending_dmas.pop(0)
                nc.sync.dma_start(dst, src)
                drip_i[0] += 1

        # ---------------- out-projection work queue --------------------
        # each unit: 2 accumulating matmuls + psum->f16 copy + DMA out
        op_queue = []
        op_flip = [0]

        ob_cur = [None]

        def emit_op_unit(act_copy=False):
            if not op_queue:
                return False
            st, hc = op_queue.pop(0)
            ssl = slice(st * 128, (st + 1) * 128)
            hsl = slice(hc * 512, (hc + 1) * 512)
            ps_o = p3.tile([128, 512], F32, tag="oc", name="ps_o")
            nc.tensor.matmul(ps_o, ao_pair[:, 0, ssl], wo_sb[:, 0, hsl],
                             start=True, stop=False)
            nc.tensor.matmul(ps_o, ao_pair[:, 1, ssl], wo_sb[:, 1, hsl],
                             start=False, stop=True)
            # 4 hc units share one wide ob tile -> ONE store per st row
            if ob_cur[0] is None:
                ob_cur[0] = obp.tile([128, HID], F16, tag="ob", name="ob")
            ob = ob_cur[0]
            # strict ACT/DVE alternation keeps the psum slots rotating at the
            # matmul rate instead of the single-engine copy rate
            if act_copy or op_flip[0] % 2 == 0:
                nc.scalar.copy(out=ob[:, hsl], in_=ps_o)
            else:
                nc.vector.tensor_copy(out=ob[:, hsl], in_=ps_o)
            if hc == 3:
                nc.sync.dma_start(outd[st], ob)
                ob_cur[0] = None
            op_flip[0] += 1
            return True

        def queue_op_chunk(c0, w):
            for st in range(c0 // 128, (c0 + w) // 128):
                for hc in range(4):
                    op_queue.append((st, hc))

        # ---------------- phase 1: projection chunk + rope -------------
        def rope_sub(t, j, raw):
            """r1/r2 matmuls + DVE combine for target t (0,1=q heads, 2=k)."""
            csl = slice(j * SC, (j + 1) * SC)
            hsl = slice((j % 2) * SC, (j % 2) * SC + SC)
            c1s, c2s = c1h[j // 2], c2h[j // 2]
            ps_uw = p2.tile([128, 2, 512], F32, tag="uw", name="ps_uw")
            nc.tensor.matmul(ps_uw[:, 0, :], r1_sb, raw,
                             start=True, stop=True)
            nc.tensor.matmul(ps_uw[:, 1, :], r2_sb, raw,
                             start=True, stop=True)
            t1 = t12p.tile([128, 512], F32, tag="t1", name="t1")
            t2 = t12p.tile([128, 512], F32, tag="t2", name="t2")
            nc.vector.tensor_mul(out=t1, in0=ps_uw[:, 0, :], in1=c1s[:, hsl])
            nc.vector.tensor_mul(out=t2, in0=ps_uw[:, 1, :], in1=c2s[:, hsl])
            dst = q_pair[:, t, csl] if t < 2 else kT[:, csl]
            nc.vector.tensor_add(out=dst, in0=t1, in1=t2)

        def p_chunk(j, fin_prev=None, warm_n=0, defer_v=False):
            sup, half = j // 2, j % 2
            xt = xt_sb[sup]
            xt8 = x8_sb[sup]
            h0 = half * 512
            DR = mybir.MatmulPerfMode.DoubleRow

            # release the previous attn chunk's out-projection work BEFORE the
            # first (possibly DMA-gated) projection matmul, bracketed by
            # already-ready units so the normalize latency is covered
            emit_op_unit()
            if fin_prev is not None:
                fin_prev()
            emit_op_unit()

            # pass B FIRST: k + q0 + q1 fp8 double-row off the small x8
            # stream, so rope/scores never wait for the bulk xt load.
            # tileA: [:,0,:] = k series; [:,1,:] = v series (pass A, later)
            tileA = p1.tile([128, 2, 512], F32, tag="mm", name="pa")
            tileB = p1.tile([128, 2, 512], F32, tag="mm", name="pq")
            for kc2 in range(NK2):
                if warm_n:
                    warm(2 * warm_n)
                elif kc2 in (2, 4, 6):
                    emit_op_unit()
                nc.tensor.matmul(
                    tileA[:, 0, :], wqk8_sb[:, kc2, :, 256:384],
                    xt8[:, kc2, :, h0:h0 + 512],
                    start=(kc2 == 0), stop=(kc2 == NK2 - 1), perf_mode=DR,
                )
                nc.tensor.matmul(
                    tileB[:, 0, :], wqk8_sb[:, kc2, :, 0:128],
                    xt8[:, kc2, :, h0:h0 + 512],
                    start=(kc2 == 0), stop=(kc2 == NK2 - 1), perf_mode=DR,
                )
                nc.tensor.matmul(
                    tileB[:, 1, :], wqk8_sb[:, kc2, :, 128:256],
                    xt8[:, kc2, :, h0:h0 + 512],
                    start=(kc2 == 0), stop=(kc2 == NK2 - 1), perf_mode=DR,
                )
            raw_k = rawp.tile([128, 512], BF16, tag="raw", name="raw_k")
            nc.scalar.copy(out=raw_k, in_=tileA[:, 0, :])
            raw_q0 = rawp.tile([128, 512], BF16, tag="raw", name="raw_q0")
            nc.scalar.copy(out=raw_q0, in_=tileB[:, 0, :])
            raw_q1 = rawp.tile([128, 512], BF16, tag="raw", name="raw_q1")
            nc.scalar.copy(out=raw_q1, in_=tileB[:, 1, :])
            rope_sub(2, j, raw_k)
            if warm_n:
                warm(4 * warm_n)
            else:
                emit_op_unit(act_copy=True)
            rope_sub(0, j, raw_q0)
            if warm_n:
                warm(4 * warm_n)
            else:
                emit_op_unit(act_copy=True)
            rope_sub(1, j, raw_q1)

            # pass A: v direct [s,d] (bf16); DVE rope combines drain behind.
            # Deferred (chunk 0): v lives in a p2 tile so tileA frees at the
            # raw_k copy and A0's scores double-buffer in p1 immediately.
            def do_v():
                if defer_v:
                    pv = p2.tile([128, 2, 512], F32, tag="uw", name="pv")
                else:
                    pv = tileA
                for kc in range(NKC):
                    if warm_n:
                        warm(warm_n)
                    elif kc in (4, 8, 12):
                        emit_op_unit()
                    for b in range(NB):
                        # ONE start=True for the whole bank: start clears the
                        # has_written bits bank-wide, so series b>0 rely on
                        # overwrite-where-clear (kc==0) + accumulate (kc>0)
                        nc.tensor.matmul(
                            pv[:, 1, b * 128:(b + 1) * 128],
                            xt[:, kc, h0 + b * 128:h0 + (b + 1) * 128],
                            wv_sb[:, kc, :],
                            start=(kc == 0 and b == 0), stop=(kc == NKC - 1),
                            skip_group_check=(b > 0),
                        )
                # v psum -> persistent [s,d] tiles
                for b in range(NB):
                    nc.scalar.copy(out=v_sd[:, j * NB + b, :],
                                   in_=pv[:, 1, b * 128:(b + 1) * 128])
            if defer_v:
                return do_v
            do_v()

        # ---------------- phase 2: attention chunk ---------------------
        # Returns a finalize closure (den -> recip -> broadcast -> normalize
        # -> queue out-projection) that the CALLER emits later, from inside
        # the next PE-busy region, so chunk boundaries never stall the PE.
        def attn_chunk(c0, w, fin_prev=None, last=False, drip_per_k=0,
                       mid_hook=None, warm_per_k=0):
            nk = (c0 + w) // 128
            # tree state: list of (level, tile) for the den pairwise sum
            tree = []

            def tree_add(entry):
                tree.append(entry)
                while len(tree) >= 2 and tree[-1][0] == tree[-2][0]:
                    l1, a = tree.pop()
                    _, b = tree.pop()
                    s_ = trp.tile([128, QH, 512], BF16, tag="tr", name="tsum")
                    with nc.allow_low_precision("bf16 den tree"):
                        nc.vector.tensor_add(
                            out=s_[:, :, :w], in0=a[:, :, :w], in1=b[:, :, :w])
                    tree.append((l1 + 1, s_))

            pend = []  # (k, v0, e) awaiting attnV
            st = {"ps_av": None}

            def emit_attnv():
                pk, pv0, pe = pend.pop(0)
                if st["ps_av"] is None:
                    st["ps_av"] = p2.tile([128, 2, 512], F32, tag="uw",
                                          name="ps_av")
                for h in range(QH):
                    nc.tensor.matmul(
                        st["ps_av"][:, h, pv0:w], v_sd[:, pk, :],
                        pe[:, h, pv0:w],
                        start=(pk == 0), stop=(pk == nk - 1),
                    )

            for k in range(nk):
                lo = 128 * k - c0  # diag block offset in chunk cols
                v0 = max(lo, 0)
                ps_s = p1.tile([128, 2, 512], F32, tag="mm", name="ps_s")
                for h in range(QH):
                    nc.tensor.matmul(
                        ps_s[:, h, v0:w], kT[:, k * 128:(k + 1) * 128],
                        q_pair[:, h, c0 + v0:c0 + w], start=True, stop=True,
                    )
                e = ep.tile([128, QH, 512], BF16, tag="e", name="e")
                nc.scalar.activation(
                    out=e[:, :, v0:w], in_=ps_s[:, :, v0:w],
                    func=mybir.ActivationFunctionType.Exp,
                    scale=SCALE / (W8S * W8S),
                )
                if v0 > 0:
                    nc.gpsimd.memset(e[:, :, 0:v0], 0.0)
                if lo > -128:  # diagonal tile: mask cols [lo, lo+128)
                    d0, d1 = max(lo, 0), min(lo + 128, w)
                    nc.vector.tensor_mul(
                        out=e[:, :, d0:d1], in0=e[:, :, d0:d1],
                        in1=tri01[:, :, d0 - lo:d1 - lo])
                tree_add((0, e))
                pend.append((k, v0, e))
                if warm_per_k:
                    warm(warm_per_k)
                if k == 0 and fin_prev is not None:
                    fin_prev()
                if len(pend) > 3 and mid_hook is None:
                    emit_attnv()
                drip(drip_per_k)
                # drain out-projection units, keeping a few in reserve to
                # cover the next chunk-boundary normalize latency
                if last:
                    if len(op_queue) > 3:
                        emit_op_unit()
                elif k < 3 or len(op_queue) > 6:
                    emit_op_unit()
            if mid_hook is not None:
                mid_hook()
            while pend:
                emit_attnv()

            # collapse leftover tree nodes (mixed levels)
            while len(tree) > 1:
                _, a = tree.pop()
                l2, b = tree.pop()
                s_ = trp.tile([128, QH, 512], BF16, tag="tr", name="tsum")
                with nc.allow_low_precision("bf16 den tree"):
                    nc.vector.tensor_add(
                        out=s_[:, :, :w], in0=a[:, :, :w], in1=b[:, :, :w])
                tree.append((l2 + 1, s_))
            acc = tree[0][1]
            ps_av = st["ps_av"]

            # den -> reciprocal -> broadcast NOW (PE den matmuls slot in right
            # behind the last attnV; recip/broadcast overlap the next phase)
            rd = dbp.tile([1, QH, 512], F32, tag="rd", name="rd")
            for h in range(QH):
                psd = p3.tile([128, 512], F32, tag="oc", name="psd")
                nc.tensor.matmul(psd[0:1, :w], ones_col, acc[:, h, :w],
                                 start=True, stop=True)
                nc.vector.reciprocal_approx_fast(
                    out=rd[:, h, :w], in_=psd[0:1, :w])
            db = dbp.tile([128, QH, 512], F32, tag="db", name="db")
            nc.gpsimd.partition_broadcast(db[:, :, :w], rd[:1, :, :w])

            def fin():
                # normalize (DVE only - no PE instructions) + release op work
                nc.vector.tensor_mul(
                    out=ao_pair[:, :, c0:c0 + w], in0=ps_av[:, :, :w],
                    in1=db[:, :, :w])
                queue_op_chunk(c0, w)

            return fin

        # ---------------- emission order -------------------------------
        warm(16)
        dv0 = p_chunk(0, warm_n=2, defer_v=True)
        queue_wo_prefetch()
        queue_sup_prefetch(1)
        fin0 = attn_chunk(0, 512, mid_hook=dv0, warm_per_k=6)
        p_chunk(1, fin_prev=fin0)
        fin1 = attn_chunk(512, 512, drip_per_k=1)
        p_chunk(2, fin_prev=fin1)
        fin2 = attn_chunk(1024, 512)
        p_chunk(3, fin_prev=fin2)
        fin3 = attn_chunk(1536, 384)
        fin4 = attn_chunk(1920, 128, fin_prev=fin3, last=True)
        emit_op_unit()
        emit_op_unit()
        fin4()
        while emit_op_unit():
            pass

    nc.finalize()
    return nc


def shard_inputs(x, cos, sin, wq, wk, wv, wo):
    x = np.asarray(x, np.float32).reshape(S, HID)
    cos = np.asarray(cos, np.float32)
    sin = np.asarray(sin, np.float32)
    wq = np.asarray(wq, np.float32)
    wk = np.asarray(wk, np.float32)
    wv = np.asarray(wv, np.float32)
    wo = np.asarray(wo, np.float32)

    xT = np.ascontiguousarray(x.T)  # [HID, S]
    # per-partition-major bf16 stream: [sup, p, kc, 1024]
    xt_blk = np.ascontiguousarray(
        xT.reshape(NKC, 128, 2, 2 * SC).transpose(2, 1, 0, 3)).astype(NPBF)
    # per-partition-major fp8 stream: [sup, p, kc2, 2, 1024]
    x8_blk = np.ascontiguousarray(
        xT.reshape(NK2, 2, 128, 2, 2 * SC).transpose(3, 2, 0, 1, 4)
    ).astype(NPF8)

    cos_h, sin_h = cos[:, :HH].T, sin[:, :HH].T       # [64, S]
    c1 = np.ascontiguousarray(
        np.concatenate([cos_h, -sin_h], axis=0)).astype(NPBF)
    c2 = np.ascontiguousarray(
        np.concatenate([sin_h, cos_h], axis=0)).astype(NPBF)

    r1 = np.zeros((HD, HD), np.float32)
    for i in range(HH // 2):
        r1[2 * i, 2 * i + 1] = -1.0
        r1[2 * i + 1, 2 * i] = 1.0
    r1[HH:, :] = r1[:HH, :]
    r2 = np.zeros((HD, HD), np.float32)
    for d in range(HH):
        r2[d, d + HH] = 1.0
        r2[d + HH, d + HH] = 1.0
    r1t = np.ascontiguousarray(r1.T).astype(NPBF)  # lhsT for out = R1 @ rhs
    r2t = np.ascontiguousarray(r2.T).astype(NPBF)

    in_maps = []
    for c in range(NCORES):
        h0 = QH * c
        kvh = h0 * NKV // NH
        wq_c = wq[h0 * HD:(h0 + QH) * HD, :]             # [256, HID]
        wk_c = wk[kvh * HD:(kvh + 1) * HD, :]            # [128, HID]
        wv_c = wv[kvh * HD:(kvh + 1) * HD, :]
        wvT_c = np.ascontiguousarray(
            wv_c.T.reshape(NKC, 128, HD).transpose(1, 0, 2)).astype(NPBF)
        wqk8_c = np.ascontiguousarray(
            (np.concatenate([wq_c, wk_c], axis=0) * W8S).T
            .reshape(NK2, 2, 128, 3 * HD).transpose(2, 0, 1, 3)).astype(NPF8)
        woT_c = np.ascontiguousarray(
            wo[:, h0 * HD:(h0 + QH) * HD].T
            .reshape(QH, 128, HID).transpose(1, 0, 2)).astype(NPBF)
        in_maps.append({
            "xt": xt_blk,
            "x8": x8_blk,
            "wvt": wvT_c,
            "wqk8": wqk8_c,
            "wot": woT_c,
            "c1": c1,
            "c2": c2,
            "r1t": r1t,
            "r2t": r2t,
        })
    return in_maps


_CACHED_NC = None


def kernel(x, cos, sin, wq, wk, wv, wo, _trace=False, _tmpdir=None):
    global _CACHED_NC
    in_maps = shard_inputs(x, cos, sin, wq, wk, wv, wo)
    if _CACHED_NC is None:
        _CACHED_NC = build_graph()
    nc = _CACHED_NC
    res = bass_utils.run_bass_kernel_spmd(
        nc, in_maps, core_ids=list(range(NCORES)),
        trace=_trace, tmpdir=_tmpdir,
    )
    total = np.zeros((S // 128, 128, HID), np.float32)
    for r in res.results:
        total += r["out"].astype(np.float32)
    out = total.reshape(1, S, HID)
    if _trace:
        return out, res
    return out


# revision 55
# speedup vs baseline: 1.0071x; 1.0071x over previous
"""Trainium2 Bass kernel for RoPE + GQA causal attention (B=1, S=2048, HID=2048,
NH=16, NKV=4, HD=128), tensor-parallel over heads across 8 NeuronCores.

Core c computes q heads {2c, 2c+1} and kv head c//2 plus the matching wo
input-dim slice; each core emits a partial [S, HID] (f16) output and the host
sums the 8 partials.

v9 design notes (vs the 181us v2 baseline):
  - 512-column compute chunks (proj j -> rope -> attn j pipelined), but x
    streams in 1024-column host-blocked supertiles so every DMA descriptor
    carries a full 2KB partition line.
  - DMA instruction count minimized (~40 total): each dma_start costs ~620ns
    of sequencer issue time regardless of size, so kc tiles ride grouped
    multi-kc DMAs and the 64 output stores merge into 16 wide per-st stores.
  - two HW-DGE queues stream in parallel (xt on sync, x8/consts on scalar);
    gpsimd issues NO DMAs (software-DGE: 600ns/issue + ~10us teardown drain).
  - per chunk, the fp8 DoubleRow k/q pass + rope run BEFORE the bf16 v pass,
    so the attention-critical path rides the small fp8 stream.
  - v is computed directly in [s, d] layout (lhsT = x tile, rhs = wv tile,
    4 psum series sharing one bank: ONE start=True, has_written handles the
    rest), removing all DMA transposes.
  - softmax den/recip/broadcast run at attn-chunk end; only the normalize
    mul + out-proj release ride the next chunk, so the PE queue head never
    blocks ready work; op-unit psum copies alternate ACT/DVE strictly.
  - warm-up dummy matmuls fill DMA-paced front slivers (HAM clock stays up).
  - carried over from v2: fp8 DoubleRow projections with W8S weight prescale,
    paired-head [128,2,512] tiles, bf16 pairwise den tree + ones-matmul,
    causal col-skip + 0/1 triangle mask, out-projection work-unit drain.
"""

import os
import sys
from contextlib import ExitStack

for _p in ("/opt/trn_rl_repo", "/root/.axon_site/_ro/trn_rl_repo"):
    if os.path.isdir(_p) and _p not in sys.path:
        sys.path.append(_p)

import ml_dtypes
import numpy as np

import concourse.bass as bass
import concourse.mybir as mybir
import concourse.tile as tile
from concourse import bacc, bass_utils

S, HID, NH, NKV, HD = 2048, 2048, 16, 4, 128
HH = HD // 2  # 64
NCORES = 8
QH = NH // NCORES  # 2 q heads per core
SCALE = float(1.0 / np.sqrt(HD))

F32 = mybir.dt.float32
BF16 = mybir.dt.bfloat16
F16 = mybir.dt.float16
F8 = mybir.dt.float8e4
NPBF = ml_dtypes.bfloat16
NPF8 = ml_dtypes.float8_e4m3fn

# q/k projections run in fp8 (DoubleRow); weights are pre-scaled by W8S on the
# host, so scores carry W8S^2 and the exp scale divides it back out
W8S = 32.0

NKC = HID // 128   # 16 contraction chunks
NK2 = HID // 256   # 8 double-row contraction chunks
SC = 512           # s-chunk width
NC = S // SC       # 4 chunks
NB = SC // 128     # 4 128-blocks per chunk
# attention chunks: (col0, width); tail kept short (last chunk 128)
ATTN_CHUNKS = [(0, 512), (512, 512), (1024, 512), (1536, 384), (1920, 128)]


def build_graph():
    nc = bacc.Bacc(trn_type="TRN2", enable_partition_id=False)

    # host-blocked streams in 1024-col supertiles, PER-PARTITION-MAJOR: each
    # partition's slice of a DMA is one long contiguous HBM run, so the DGE
    # emits 8-16KB descriptors instead of 2KB ones (expansion rate was the
    # front-fill bottleneck)
    xt_d = nc.dram_tensor("xt", [2, 128, NKC, 2 * SC], BF16,
                          kind="ExternalInput")
    x8_d = nc.dram_tensor("x8", [2, 128, NK2, 2, 2 * SC], F8,
                          kind="ExternalInput")
    wv_d = nc.dram_tensor("wvt", [128, NKC, HD], BF16, kind="ExternalInput")
    wqk8_d = nc.dram_tensor("wqk8", [128, NK2, 2, 3 * HD], F8,
                            kind="ExternalInput")
    wo_d = nc.dram_tensor("wot", [128, QH, HID], BF16, kind="ExternalInput")
    c1d = nc.dram_tensor("c1", [HD, S], BF16, kind="ExternalInput")
    c2d = nc.dram_tensor("c2", [HD, S], BF16, kind="ExternalInput")
    r1d = nc.dram_tensor("r1t", [HD, HD], BF16, kind="ExternalInput")
    r2d = nc.dram_tensor("r2t", [HD, HD], BF16, kind="ExternalInput")
    # blocked output: [st, 128, 2048] f16 (one wide store per 128-row block)
    outd = nc.dram_tensor("out", [S // 128, 128, HID], F16,
                          kind="ExternalOutput")


    with tile.TileContext(nc) as tc, ExitStack() as ctx:
        # ---------- pools
        consts = ctx.enter_context(tc.tile_pool(name="consts", bufs=1))
        persist = ctx.enter_context(tc.tile_pool(name="persist", bufs=1))
        xtp = ctx.enter_context(tc.tile_pool(name="xtp", bufs=2))
        x8p = ctx.enter_context(tc.tile_pool(name="x8p", bufs=2))
        rawp = ctx.enter_context(tc.tile_pool(name="rawp", bufs=3))
        t12p = ctx.enter_context(tc.tile_pool(name="t12p", bufs=1))
        ep = ctx.enter_context(tc.tile_pool(name="ep", bufs=6))
        trp = ctx.enter_context(tc.tile_pool(name="trp", bufs=5))
        obp = ctx.enter_context(tc.tile_pool(name="obp", bufs=3))
        dbp = ctx.enter_context(tc.tile_pool(name="dbp", bufs=2))
        # PSUM: 4 + 2 + 2 = 8 banks exactly
        p1 = ctx.enter_context(
            tc.tile_pool(name="p1", bufs=2, space="PSUM"))   # [128,2,512]f32 x2
        p2 = ctx.enter_context(
            tc.tile_pool(name="p2", bufs=1, space="PSUM"))   # [128,2,512]f32 x1
        p3 = ctx.enter_context(
            tc.tile_pool(name="p3", bufs=2, space="PSUM"))   # [128,512]f32 x2

        # ---------- persistent SBUF
        q_pair = persist.tile([128, QH, S], BF16, tag="q_pair")
        kT = persist.tile([128, S], BF16, tag="kT")
        v_sd = persist.tile([128, S // 128, HD], BF16, tag="v_sd")
        ao_pair = persist.tile([128, QH, S], BF16, tag="ao_pair")
        wv_sb = persist.tile([128, NKC, HD], BF16, tag="wv_sb")
        wqk8_sb = persist.tile([128, NK2, 2, 3 * HD], F8, tag="wqk8_sb")
        wo_sb = persist.tile([128, QH, HID], BF16, tag="wo_sb")
        c1h = [persist.tile([128, S // 2], BF16, tag=f"c1_{i}",
                            name=f"c1_{i}") for i in range(2)]
        c2h = [persist.tile([128, S // 2], BF16, tag=f"c2_{i}",
                            name=f"c2_{i}") for i in range(2)]

        r1_sb = consts.tile([128, 128], BF16)
        r2_sb = consts.tile([128, 128], BF16)
        ones_col = consts.tile([128, 1], BF16)
        nc.vector.memset(ones_col, 1.0)
        # 0/1 lower-triangle keep mask (keep where col >= row), both heads
        tri01 = consts.tile([128, QH, 128], BF16)
        nc.gpsimd.memset(tri01, 1.0)
        nc.gpsimd.affine_select(
            out=tri01,
            in_=tri01,
            compare_op=mybir.AluOpType.is_ge,
            fill=0.0,
            base=0,
            pattern=[[0, QH], [1, 128]],  # value = col
            channel_multiplier=-1,        # - row
        )

        # ---------- x supertile SBUF tiles + initial DMAs (sup 0 + weights)
        xt_sb = {}
        x8_sb = {}

        def alloc_sup(sup):
            xt_sb[sup] = xtp.tile([128, NKC, 2 * SC], BF16, tag="xt",
                                  name=f"xt{sup}")
            x8_sb[sup] = x8p.tile([128, NK2, 2, 2 * SC], F8, tag="x8",
                                  name=f"x8{sup}")

        # Every dma_start costs ~620ns of sequencer issue time regardless of
        # size, so DMAs are batched into as few instructions as possible.
        # gpsimd additionally must issue NO DMAs (software-DGE + 10us drain).
        alloc_sup(0)
        # scalar feeds the q/k pass (x8) plus rope consts; sync feeds the
        # (now later) v pass. Two DGE queues stream in parallel.
        nc.scalar.dma_start(r1_sb, r1d[:, :])
        nc.scalar.dma_start(r2_sb, r2d[:, :])
        nc.scalar.dma_start(wqk8_sb, wqk8_d[:, :, :, :])
        nc.sync.dma_start(c1h[0], c1d[:, 0:1024])
        nc.scalar.dma_start(x8_sb[0][:, 0:4, :, :], x8_d[0, :, 0:4, :, :])
        nc.sync.dma_start(xt_sb[0][:, 0:8, :], xt_d[0, :, 0:8, :])
        nc.scalar.dma_start(x8_sb[0][:, 4:8, :, :], x8_d[0, :, 4:8, :, :])
        nc.scalar.dma_start(c2h[0], c2d[:, 0:1024])
        nc.sync.dma_start(xt_sb[0][:, 8:16, :], xt_d[0, :, 8:16, :])
        nc.scalar.dma_start(wv_sb, wv_d[:, :, :])
        nc.scalar.dma_start(wo_sb, wo_d[:, :, :])

        # ---------- PE warm-up: dummy matmuls that fill DMA-paced slivers in
        # the front so the HAM clock ramps to 2.4GHz before the dense phase
        warm_tile = [None]

        def warm(n):
            if warm_tile[0] is None:
                warm_tile[0] = p3.tile([128, 512], F32, tag="oc", name="warm")
            for _ in range(n):
                nc.tensor.matmul(warm_tile[0][:, 0:128], r1_sb, r1_sb,
                                 start=True, stop=True)

        # ---------- dribbled prefetch queue (issued on sync only)
        pending_dmas = []
        drip_i = [0]

        def queue_sup_prefetch(sup):
            alloc_sup(sup)
            for g in range(4):
                pending_dmas.append((xt_sb[sup][:, 4 * g:4 * g + 4, :],
                                     xt_d[sup, :, 4 * g:4 * g + 4, :]))
                if g % 2 == 1:
                    k0 = 2 * g - 2
                    pending_dmas.append((x8_sb[sup][:, k0:k0 + 4, :, :],
                                         x8_d[sup, :, k0:k0 + 4, :, :]))

        def queue_wo_prefetch():
            pending_dmas.append((c1h[1], c1d[:, 1024:2048]))
            pending_dmas.append((c2h[1], c2d[:, 1024:2048]))

        def drip(n):
            for _ in range(n):
                if not pending_dmas:
                    return
                dst, src = pending_dmas.pop(0)
                nc.sync.dma_start(dst, src)
                drip_i[0] += 1

        # ---------------- out-projection work queue --------------------
        # each unit: 2 accumulating matmuls + psum->f16 copy + DMA out
        op_queue = []
        op_flip = [0]

        ob_cur = [None]

        def emit_op_unit(act_copy=False):
            if not op_queue:
                return False
            st, hc = op_queue.pop(0)
            ssl = slice(st * 128, (st + 1) * 128)
            hsl = slice(hc * 512, (hc + 1) * 512)
            ps_o = p3.tile([128, 512], F32, tag="oc", name="ps_o")
            nc.tensor.matmul(ps_o, ao_pair[:, 0, ssl], wo_sb[:, 0, hsl],
                             start=True, stop=False)
            nc.tensor.matmul(ps_o, ao_pair[:, 1, ssl], wo_sb[:, 1, hsl],
                             start=False, stop=True)
            # 4 hc units share one wide ob tile -> ONE store per st row
            if ob_cur[0] is None:
                ob_cur[0] = obp.tile([128, HID], F16, tag="ob", name="ob")
            ob = ob_cur[0]
            # strict ACT/DVE alternation keeps the psum slots rotating at the
            # matmul rate instead of the single-engine copy rate
            if act_copy or op_flip[0] % 2 == 0:
                nc.scalar.copy(out=ob[:, hsl], in_=ps_o)
            else:
                nc.vector.tensor_copy(out=ob[:, hsl], in_=ps_o)
            if hc == 3:
                nc.sync.dma_start(outd[st], ob)
                ob_cur[0] = None
            op_flip[0] += 1
            return True

        def queue_op_chunk(c0, w):
            for st in range(c0 // 128, (c0 + w) // 128):
                for hc in range(4):
                    op_queue.append((st, hc))

        # ---------------- phase 1: projection chunk + rope -------------
        def rope_sub(t, j, raw):
            """r1/r2 matmuls + DVE combine for target t (0,1=q heads, 2=k)."""
            csl = slice(j * SC, (j + 1) * SC)
            hsl = slice((j % 2) * SC, (j % 2) * SC + SC)
            c1s, c2s = c1h[j // 2], c2h[j // 2]
            ps_uw = p2.tile([128, 2, 512], F32, tag="uw", name="ps_uw")
            nc.tensor.matmul(ps_uw[:, 0, :], r1_sb, raw,
                             start=True, stop=True)
            nc.tensor.matmul(ps_uw[:, 1, :], r2_sb, raw,
                             start=True, stop=True)
            t1 = t12p.tile([128, 512], F32, tag="t1", name="t1")
            t2 = t12p.tile([128, 512], F32, tag="t2", name="t2")
            nc.vector.tensor_mul(out=t1, in0=ps_uw[:, 0, :], in1=c1s[:, hsl])
            nc.vector.tensor_mul(out=t2, in0=ps_uw[:, 1, :], in1=c2s[:, hsl])
            dst = q_pair[:, t, csl] if t < 2 else kT[:, csl]
            nc.vector.tensor_add(out=dst, in0=t1, in1=t2)

        def p_chunk(j, fin_prev=None, warm_n=0, defer_v=False):
            sup, half = j // 2, j % 2
            xt = xt_sb[sup]
            xt8 = x8_sb[sup]
            h0 = half * 512
            DR = mybir.MatmulPerfMode.DoubleRow

            # release the previous attn chunk's out-projection work BEFORE the
            # first (possibly DMA-gated) projection matmul, bracketed by
            # already-ready units so the normalize latency is covered
            emit_op_unit()
            if fin_prev is not None:
                fin_prev()
            emit_op_unit()

            # pass B FIRST: k + q0 + q1 fp8 double-row off the small x8
            # stream, so rope/scores never wait for the bulk xt load.
            # tileA: [:,0,:] = k series; [:,1,:] = v series (pass A, later)
            tileA = p1.tile([128, 2, 512], F32, tag="mm", name="pa")
            tileB = p1.tile([128, 2, 512], F32, tag="mm", name="pq")
            for kc2 in range(NK2):
                if warm_n:
                    warm(2 * warm_n)
                elif kc2 in (2, 4, 6):
                    emit_op_unit()
                nc.tensor.matmul(
                    tileA[:, 0, :], wqk8_sb[:, kc2, :, 256:384],
                    xt8[:, kc2, :, h0:h0 + 512],
                    start=(kc2 == 0), stop=(kc2 == NK2 - 1), perf_mode=DR,
                )
                nc.tensor.matmul(
                    tileB[:, 0, :], wqk8_sb[:, kc2, :, 0:128],
                    xt8[:, kc2, :, h0:h0 + 512],
                    start=(kc2 == 0), stop=(kc2 == NK2 - 1), perf_mode=DR,
                )
                nc.tensor.matmul(
                    tileB[:, 1, :], wqk8_sb[:, kc2, :, 128:256],
                    xt8[:, kc2, :, h0:h0 + 512],
                    start=(kc2 == 0), stop=(kc2 == NK2 - 1), perf_mode=DR,
                )
            raw_k = rawp.tile([128, 512], BF16, tag="raw", name="raw_k")
            nc.scalar.copy(out=raw_k, in_=tileA[:, 0, :])
            raw_q0 = rawp.tile([128, 512], BF16, tag="raw", name="raw_q0")
            nc.scalar.copy(out=raw_q0, in_=tileB[:, 0, :])
            raw_q1 = rawp.tile([128, 512], BF16, tag="raw", name="raw_q1")
            nc.scalar.copy(out=raw_q1, in_=tileB[:, 1, :])
            rope_sub(2, j, raw_k)
            if warm_n:
                warm(4 * warm_n)
            else:
                emit_op_unit(act_copy=True)
            rope_sub(0, j, raw_q0)
            if warm_n:
                warm(4 * warm_n)
            else:
                emit_op_unit(act_copy=True)
            rope_sub(1, j, raw_q1)

            # pass A: v direct [s,d] (bf16); DVE rope combines drain behind
            def do_v():
                for kc in range(NKC):
                    if warm_n:
                        warm(warm_n)
                    elif kc in (4, 8, 12):
                        emit_op_unit()
                    for b in range(NB):
                        # ONE start=True for the whole bank: start clears the
                        # has_written bits bank-wide, so series b>0 rely on
                        # overwrite-where-clear (kc==0) + accumulate (kc>0)
                        nc.tensor.matmul(
                            tileA[:, 1, b * 128:(b + 1) * 128],
                            xt[:, kc, h0 + b * 128:h0 + (b + 1) * 128],
                            wv_sb[:, kc, :],
                            start=(kc == 0 and b == 0), stop=(kc == NKC - 1),
                            skip_group_check=(b > 0),
                        )
                # v psum -> persistent [s,d] tiles
                for b in range(NB):
                    nc.scalar.copy(out=v_sd[:, j * NB + b, :],
                                   in_=tileA[:, 1, b * 128:(b + 1) * 128])
            if defer_v:
                return do_v
            do_v()

        # ---------------- phase 2: attention chunk ---------------------
        # Returns a finalize closure (den -> recip -> broadcast -> normalize
        # -> queue out-projection) that the CALLER emits later, from inside
        # the next PE-busy region, so chunk boundaries never stall the PE.
        def attn_chunk(c0, w, fin_prev=None, last=False, drip_per_k=0,
                       mid_hook=None):
            nk = (c0 + w) // 128
            # tree state: list of (level, tile) for the den pairwise sum
            tree = []

            def tree_add(entry):
                tree.append(entry)
                while len(tree) >= 2 and tree[-1][0] == tree[-2][0]:
                    l1, a = tree.pop()
                    _, b = tree.pop()
                    s_ = trp.tile([128, QH, 512], BF16, tag="tr", name="tsum")
                    with nc.allow_low_precision("bf16 den tree"):
                        nc.vector.tensor_add(
                            out=s_[:, :, :w], in0=a[:, :, :w], in1=b[:, :, :w])
                    tree.append((l1 + 1, s_))

            pend = []  # (k, v0, e) awaiting attnV
            st = {"ps_av": None}

            def emit_attnv():
                pk, pv0, pe = pend.pop(0)
                if st["ps_av"] is None:
                    st["ps_av"] = p2.tile([128, 2, 512], F32, tag="uw",
                                          name="ps_av")
                for h in range(QH):
                    nc.tensor.matmul(
                        st["ps_av"][:, h, pv0:w], v_sd[:, pk, :],
                        pe[:, h, pv0:w],
                        start=(pk == 0), stop=(pk == nk - 1),
                    )

            for k in range(nk):
                lo = 128 * k - c0  # diag block offset in chunk cols
                v0 = max(lo, 0)
                ps_s = p1.tile([128, 2, 512], F32, tag="mm", name="ps_s")
                for h in range(QH):
                    nc.tensor.matmul(
                        ps_s[:, h, v0:w], kT[:, k * 128:(k + 1) * 128],
                        q_pair[:, h, c0 + v0:c0 + w], start=True, stop=True,
                    )
                e = ep.tile([128, QH, 512], BF16, tag="e", name="e")
                nc.scalar.activation(
                    out=e[:, :, v0:w], in_=ps_s[:, :, v0:w],
                    func=mybir.ActivationFunctionType.Exp,
                    scale=SCALE / (W8S * W8S),
                )
                if v0 > 0:
                    nc.gpsimd.memset(e[:, :, 0:v0], 0.0)
                if lo > -128:  # diagonal tile: mask cols [lo, lo+128)
                    d0, d1 = max(lo, 0), min(lo + 128, w)
                    nc.vector.tensor_mul(
                        out=e[:, :, d0:d1], in0=e[:, :, d0:d1],
                        in1=tri01[:, :, d0 - lo:d1 - lo])
                tree_add((0, e))
                pend.append((k, v0, e))
                if k == 0 and fin_prev is not None:
                    fin_prev()
                if len(pend) > 3:
                    emit_attnv()
                drip(drip_per_k)
                # drain out-projection units, keeping a few in reserve to
                # cover the next chunk-boundary normalize latency
                if last:
                    if len(op_queue) > 3:
                        emit_op_unit()
                elif k < 3 or len(op_queue) > 6:
                    emit_op_unit()
            if mid_hook is not None:
                mid_hook()
            while pend:
                emit_attnv()

            # collapse leftover tree nodes (mixed levels)
            while len(tree) > 1:
                _, a = tree.pop()
                l2, b = tree.pop()
                s_ = trp.tile([128, QH, 512], BF16, tag="tr", name="tsum")
                with nc.allow_low_precision("bf16 den tree"):
                    nc.vector.tensor_add(
                        out=s_[:, :, :w], in0=a[:, :, :w], in1=b[:, :, :w])
                tree.append((l2 + 1, s_))
            acc = tree[0][1]
            ps_av = st["ps_av"]

            # den -> reciprocal -> broadcast NOW (PE den matmuls slot in right
            # behind the last attnV; recip/broadcast overlap the next phase)
            rd = dbp.tile([1, QH, 512], F32, tag="rd", name="rd")
            for h in range(QH):
                psd = p3.tile([128, 512], F32, tag="oc", name="psd")
                nc.tensor.matmul(psd[0:1, :w], ones_col, acc[:, h, :w],
                                 start=True, stop=True)
                nc.vector.reciprocal_approx_fast(
                    out=rd[:, h, :w], in_=psd[0:1, :w])
            db = dbp.tile([128, QH, 512], F32, tag="db", name="db")
            nc.gpsimd.partition_broadcast(db[:, :, :w], rd[:1, :, :w])

            def fin():
                # normalize (DVE only - no PE instructions) + release op work
                nc.vector.tensor_mul(
                    out=ao_pair[:, :, c0:c0 + w], in0=ps_av[:, :, :w],
                    in1=db[:, :, :w])
                queue_op_chunk(c0, w)

            return fin

        # ---------------- emission order -------------------------------
        warm(16)
        dv0 = p_chunk(0, warm_n=2, defer_v=True)
        queue_wo_prefetch()
        queue_sup_prefetch(1)
        fin0 = attn_chunk(0, 512, mid_hook=dv0)
        p_chunk(1, fin_prev=fin0)
        fin1 = attn_chunk(512, 512, drip_per_k=1)
        p_chunk(2, fin_prev=fin1)
        fin2 = attn_chunk(1024, 512)
        p_chunk(3, fin_prev=fin2)
        fin3 = attn_chunk(1536, 384)
        fin4 = attn_chunk(1920, 128, fin_prev=fin3, last=True)
        emit_op_unit()
        emit_op_unit()
        fin4()
        while emit_op_unit():
            pass

    nc.finalize()
    return nc


def shard_inputs(x, cos, sin, wq, wk, wv, wo):
    x = np.asarray(x, np.float32).reshape(S, HID)
    cos = np.asarray(cos, np.float32)
    sin = np.asarray(sin, np.float32)
    wq = np.asarray(wq, np.float32)
    wk = np.asarray(wk, np.float32)
    wv = np.asarray(wv, np.float32)
    wo = np.asarray(wo, np.float32)

    xT = np.ascontiguousarray(x.T)  # [HID, S]
    # per-partition-major bf16 stream: [sup, p, kc, 1024]
    xt_blk = np.ascontiguousarray(
        xT.reshape(NKC, 128, 2, 2 * SC).transpose(2, 1, 0, 3)).astype(NPBF)
    # per-partition-major fp8 stream: [sup, p, kc2, 2, 1024]
    x8_blk = np.ascontiguousarray(
        xT.reshape(NK2, 2, 128, 2, 2 * SC).transpose(3, 2, 0, 1, 4)
    ).astype(NPF8)

    cos_h, sin_h = cos[:, :HH].T, sin[:, :HH].T       # [64, S]
    c1 = np.ascontiguousarray(
        np.concatenate([cos_h, -sin_h], axis=0)).astype(NPBF)
    c2 = np.ascontiguousarray(
        np.concatenate([sin_h, cos_h], axis=0)).astype(NPBF)

    r1 = np.zeros((HD, HD), np.float32)
    for i in range(HH // 2):
        r1[2 * i, 2 * i + 1] = -1.0
        r1[2 * i + 1, 2 * i] = 1.0
    r1[HH:, :] = r1[:HH, :]
    r2 = np.zeros((HD, HD), np.float32)
    for d in range(HH):
        r2[d, d + HH] = 1.0
        r2[d + HH, d + HH] = 1.0
    r1t = np.ascontiguousarray(r1.T).astype(NPBF)  # lhsT for out = R1 @ rhs
    r2t = np.ascontiguousarray(r2.T).astype(NPBF)

    in_maps = []
    for c in range(NCORES):
        h0 = QH * c
        kvh = h0 * NKV // NH
        wq_c = wq[h0 * HD:(h0 + QH) * HD, :]             # [256, HID]
        wk_c = wk[kvh * HD:(kvh + 1) * HD, :]            # [128, HID]
        wv_c = wv[kvh * HD:(kvh + 1) * HD, :]
        wvT_c = np.ascontiguousarray(
            wv_c.T.reshape(NKC, 128, HD).transpose(1, 0, 2)).astype(NPBF)
        wqk8_c = np.ascontiguousarray(
            (np.concatenate([wq_c, wk_c], axis=0) * W8S).T
            .reshape(NK2, 2, 128, 3 * HD).transpose(2, 0, 1, 3)).astype(NPF8)
        woT_c = np.ascontiguousarray(
            wo[:, h0 * HD:(h0 + QH) * HD].T
            .reshape(QH, 128, HID).transpose(1, 0, 2)).astype(NPBF)
        in_maps.append({
            "xt": xt_blk,
            "x8": x8_blk,
            "wvt": wvT_c,
            "wqk8": wqk8_c,
            "wot": woT_c,
            "c1": c1,
            "c2": c2,
            "r1t": r1t,
            "r2t": r2t,
        })
    return in_maps


_CACHED_NC = None


def kernel(x, cos, sin, wq, wk, wv, wo, _trace=False, _tmpdir=None):
    global _CACHED_NC
    in_maps = shard_inputs(x, cos, sin, wq, wk, wv, wo)
    if _CACHED_NC is None:
        _CACHED_NC = build_graph()
    nc = _CACHED_NC
    res = bass_utils.run_bass_kernel_spmd(
        nc, in_maps, core_ids=list(range(NCORES)),
        trace=_trace, tmpdir=_tmpdir,
    )
    total = np.zeros((S // 128, 128, HID), np.float32)
    for r in res.results:
        total += r["out"].astype(np.float32)
    out = total.reshape(1, S, HID)
    if _trace:
        return out, res
    return out
